# revision 1
# baseline (speedup 1.0000x reference)
"""CSWin attention Trainium2 kernel.

Shapes (hardcoded): B=8, H=W=64, N=4096, C=512, 8 heads (4 horizontal-stripe,
4 vertical-stripe), head_dim=64, stripe width SPLIT=8.

Sharding: data-parallel over batch B across the 8 NeuronCores (1 image/core).

On-chip strategy (per core, all matmuls bf16 with fp32 PSUM accumulation):
  - x [4096, 512] -> xT [512, 4096] (PE transposes, copies split ACT/DVE),
    channel-major.
  - qkvT [1536, 4096] = WqkvT @ xT (+bias folded into the PSUM->SBUF
    tensor_scalar_add copy).  v-half head channels are written in
    column-major token order so vertical stripes are contiguous too.
  - attention runs as two interleaved independent streams (h-half /
    v-half) of head-pairs, software-pipelined one pair ahead:
      * LePE depthwise conv: 9 K=128 diagonal matmuls with shifted
        2-level-AP rhs windows, accumulating v + conv(v) (+bias in the
        copy-out) in PSUM for a whole head pair at once;
      * scoresT = k-stationary matmul (head pairs packed onto disjoint
        PE row groups), exp on ScalarE straight out of PSUM (no
        max-subtraction needed: scores ~ N(0,1));
      * AV matmul with a ones-augmented transposed v_lepe so PSUM row 64
        is the softmax denominator for free;
      * normalization: DVE reciprocal of the denominator row, GPSIMD
        partition_broadcast + multiply scattering into concatT.
  - final proj: token-major PSUM matmuls from concatT + WprojT, bias via
    K=1 ones matmul, DMA out.
"""

import os
import numpy as np

import concourse.bass as bass
import concourse.bacc as bacc
import concourse.mybir as mybir
from concourse import bass_utils
from concourse.tile import TileContext
from concourse.masks import make_identity

F32 = mybir.dt.float32
BF16 = mybir.dt.bfloat16

B = 8
H = 64
W = 64
N = H * W          # 4096
C = 512
NH = 8             # heads
HD = 64            # head dim
SP = 8             # stripe width
NS = 8             # stripes per direction
SCALE = HD ** -0.5

_CACHE = {}


def _build_nc():
    nc = bacc.Bacc("TRN2", target_bir_lowering=False, debug=False)

    x_d = nc.dram_tensor("x", (N, C), F32, kind="ExternalInput").ap()
    wqkv_d = nc.dram_tensor("wqkv", (3 * C, C), F32, kind="ExternalInput").ap()
    bqkv_d = nc.dram_tensor("bqkv", (3 * C,), F32, kind="ExternalInput").ap()
    wproj_d = nc.dram_tensor("wproj", (C, C), F32, kind="ExternalInput").ap()
    bproj_d = nc.dram_tensor("bproj", (C,), F32, kind="ExternalInput").ap()
    lhw_d = nc.dram_tensor("lepe_h_w", (3, 3, 1, HD), F32, kind="ExternalInput").ap()
    lhb_d = nc.dram_tensor("lepe_h_b", (HD,), F32, kind="ExternalInput").ap()
    lvw_d = nc.dram_tensor("lepe_v_w", (3, 3, 1, HD), F32, kind="ExternalInput").ap()
    lvb_d = nc.dram_tensor("lepe_v_b", (HD,), F32, kind="ExternalInput").ap()
    y_d = nc.dram_tensor("y", (N, C), F32, kind="ExternalOutput").ap()

    with TileContext(nc) as tc:
        _emit(nc, tc, x_d, wqkv_d, bqkv_d, wproj_d, bproj_d,
              lhw_d, lhb_d, lvw_d, lvb_d, y_d)
    nc.compile()
    return nc


def _emit(nc, tc, x_d, wqkv_d, bqkv_d, wproj_d, bproj_d,
          lhw_d, lhb_d, lvw_d, lvb_d, y_d):
    import contextlib
    ctx = contextlib.ExitStack()
    with ctx:
        persist = ctx.enter_context(tc.tile_pool(name="persist", bufs=1))
        qkv_pool = ctx.enter_context(tc.tile_pool(name="qkvT", bufs=1))

        from concourse import library_config
        nc.gpsimd.load_library(library_config.proxy)

        # ---------------- constants / weights prep ----------------
        # identity [128, 128] bf16 for 128-row transposes
        id128 = persist.tile([128, 128], BF16, tag="id128")
        make_identity(nc, id128)
        # identity [64, 64] bf16 (rhs for the tiny [9, 64] weight transpose)
        id64 = persist.tile([64, 64], BF16, tag="id64")
        make_identity(nc, id64)


        # ---------------- xT, then qkvT [1536, 4096] ----------------
        # qkvT: 12 tiles [128, 4096]; tile jt holds channels
        # [128*jt, 128*jt+128): jt 0-3: q (heads 0-7), 4-7: k, 8-11: v.
        # Within each group tiles 0-1 = h-half heads (row-major tokens),
        # 2-3 = v-half heads (column-major token order t' = x*64 + y).
        qkvT = [qkv_pool.tile([128, N], BF16, name=f"qkvT{jt}", tag=f"qkvT{jt}") for jt in range(12)]
        with tc.tile_pool(name="xT", bufs=1) as xT_pool:
            xT = [xT_pool.tile([128, N], BF16, name=f"xT{cc}", tag=f"xT{cc}") for cc in range(4)]
            with tc.tile_pool(name="xload", bufs=2) as xload, \
                 tc.tile_pool(name="xt_psum", bufs=4, space="PSUM") as xt_psum:
                for tg in range(8):
                    rows_bf = []
                    for j in range(4):
                        tt = tg * 4 + j
                        xrow = xload.tile([128, C], F32, tag=f"xrow{j}")
                        nc.sync.dma_start(
                            out=xrow, in_=x_d[tt * 128:(tt + 1) * 128, :])
                        xrow_bf = xload.tile([128, C], BF16, tag=f"xrow_bf{j}")
                        nc.vector.tensor_copy(xrow_bf, xrow)
                        rows_bf.append(xrow_bf)
                    for cc in range(4):
                        ps = xt_psum.tile([128, 512], BF16, tag="xps")
                        for j in range(4):
                            nc.tensor.transpose(
                                ps[:, j * 128:(j + 1) * 128],
                                rows_bf[j][:, cc * 128:(cc + 1) * 128], id128)
                        if cc % 2 == 0:
                            nc.scalar.activation(
                                xT[cc][:, tg * 512:(tg + 1) * 512], ps,
                                mybir.ActivationFunctionType.Copy)
                        else:
                            nc.vector.tensor_copy(
                                xT[cc][:, tg * 512:(tg + 1) * 512], ps)

                # --- load + cast + transpose Wqkv -> WqkvT [c, j]: 4 tiles [128, 1536]
                # (casts on DVE; PSUM->SBUF copies on the otherwise-idle ScalarE)
                wqkvT = [persist.tile([128, 3 * C], BF16, name=f"wqkvT{cc}", tag=f"wqkvT{cc}")
                         for cc in range(4)]
                for jg in range(3):
                    rows_bf = []
                    for j in range(4):
                        jt = jg * 4 + j
                        wrow = xload.tile([128, C], F32, tag=f"xrow{j}")
                        nc.sync.dma_start(
                            out=wrow, in_=wqkv_d[jt * 128:(jt + 1) * 128, :])
                        wrow_bf = xload.tile([128, C], BF16, tag=f"xrow_bf{j}")
                        nc.vector.tensor_copy(wrow_bf, wrow)
                        rows_bf.append(wrow_bf)
                    for cc in range(4):
                        ps = xt_psum.tile([128, 512], BF16, tag="xps")
                        for j in range(4):
                            nc.tensor.transpose(
                                ps[:, j * 128:(j + 1) * 128],
                                rows_bf[j][:, cc * 128:(cc + 1) * 128], id128)
                        if cc % 2 == 0:
                            nc.scalar.activation(
                                wqkvT[cc][:, jg * 512:(jg + 1) * 512], ps,
                                mybir.ActivationFunctionType.Copy)
                        else:
                            nc.vector.tensor_copy(
                                wqkvT[cc][:, jg * 512:(jg + 1) * 512], ps)

                # --- Wproj -> WprojT [f, e]: 4 tiles [128, 512]
                wprojT = [persist.tile([128, C], BF16, name=f"wprojT{fc}", tag=f"wprojT{fc}")
                          for fc in range(4)]
                rows_bf = []
                for et in range(4):
                    wrow = xload.tile([128, C], F32, tag=f"xrow{et}")
                    nc.sync.dma_start(out=wrow, in_=wproj_d[et * 128:(et + 1) * 128, :])
                    wrow_bf = xload.tile([128, C], BF16, tag=f"xrow_bf{et}")
                    nc.vector.tensor_copy(wrow_bf, wrow)
                    rows_bf.append(wrow_bf)
                for fc in range(4):
                    ps = xt_psum.tile([128, 512], BF16, tag="xps")
                    for et in range(4):
                        nc.tensor.transpose(
                            ps[:, et * 128:(et + 1) * 128],
                            rows_bf[et][:, fc * 128:(fc + 1) * 128], id128)
                    nc.scalar.activation(
                        wprojT[fc], ps, mybir.ActivationFunctionType.Copy)

                # --- biases ---
                # bqkv per-partition: [128, 12] (partition p, col jt) = bqkv[jt*128+p]
                bqkv_sb = persist.tile([128, 12], F32, tag="bqkv")
                nc.sync.dma_start(out=bqkv_sb, in_=bqkv_d.rearrange("(a p) -> p a", p=128))
                # bproj as a bf16 row [1, 512] (K=1 matmul rhs)
                bproj_f32 = xload.tile([1, C], F32, tag="xrow0", name="bproj_f32")
                nc.sync.dma_start(out=bproj_f32, in_=bproj_d.rearrange("(a e) -> a e", a=1))
                bproj_sb = persist.tile([1, C], BF16, tag="bproj")
                nc.vector.tensor_copy(bproj_sb, bproj_f32)
                ones_row = persist.tile([1, 128], BF16, tag="ones_row")
                nc.vector.memset(ones_row, 1.0)
                # lepe biases [128, 1] (duplicated across both 64-row halves so one
                # op covers a head pair)
                lepe_b = []
                for name, d in (("lhb", lhb_d), ("lvb", lvb_d)):
                    t = persist.tile([128, 1], F32, name=name, tag=name)
                    nc.sync.dma_start(out=t[0:64, :], in_=d.rearrange("(p a) -> p a", a=1))
                    nc.sync.dma_start(out=t[64:128, :], in_=d.rearrange("(p a) -> p a", a=1))
                    lepe_b.append(t)

                # --- LePE diag weights ---
                # load [9, 64], transpose to wT [64, 9], duplicate to [128, 9],
                # then diag tiles [128, 64] (both 64-row halves hold the same diag).
                # diags[half][k] for taps k=0..8 ((dr,dc) row-major); center (k=4)
                # gets I added.
                diags = []
                for half, wsrc in ((0, lhw_d), (1, lvw_d)):
                    w9 = xload.tile([9, 64], F32, tag="w9")
                    nc.sync.dma_start(out=w9, in_=wsrc.rearrange("a b c d -> (a b c) d"))
                    w9_bf = xload.tile([9, 64], BF16, tag="w9bf")
                    nc.vector.tensor_copy(w9_bf, w9)
                    ps = xt_psum.tile([64, 9], BF16, tag="wTps", bufs=1)
                    nc.tensor.transpose(ps, w9_bf, id64[0:9, 0:9])
                    wT = persist.tile([128, 9], F32, tag=f"wT{half}")
                    nc.vector.tensor_copy(wT[0:64, :], ps)
                    nc.sync.dma_start(out=wT[64:128, :], in_=wT[0:64, :])
                    dh = []
                    nh = []
                    for k in range(9):
                        # tap index k -> (dr, dc); weight index depends on half:
                        # half 0 (horizontal): local (dr, dc) = (dy, dx) -> w[dy, dx]
                        # half 1 (vertical):   local (dr, dc) = (dx, dy) -> w[dy=dc, dx=dr]
                        dr, dc = k // 3 - 1, k % 3 - 1
                        if half == 0:
                            wi = (dr + 1) * 3 + (dc + 1)
                        else:
                            wi = (dc + 1) * 3 + (dr + 1)
                        # [128, 128] diagonal covering a head PAIR (weights repeat
                        # every 64 channels via the duplicated wT halves)
                        dt = persist.tile([128, 128], BF16, tag=f"diag{half}_{k}")
                        nc.vector.tensor_scalar_mul(dt, id128, wT[:, wi:wi + 1])
                        if k == 4:
                            nc.vector.tensor_add(dt, dt, id128)
                        dh.append(dt)
                    diags.append(dh)


            with tc.tile_pool(name="qkv_psum", bufs=4, space="PSUM") as qkv_psum:
                for jt in range(12):
                    vhalf = (jt % 4) >= 2
                    for nt in range(8):
                        ps = qkv_psum.tile([128, 512], F32, tag="qkvps")
                        for cc in range(4):
                            nc.tensor.matmul(
                                ps, wqkvT[cc][:, jt * 128:(jt + 1) * 128],
                                xT[cc][:, nt * 512:(nt + 1) * 512],
                                start=(cc == 0), stop=(cc == 3))
                        if vhalf:
                            # scatter token chunk (rows y in [8nt, 8nt+8),
                            # all x) into column-major: addr = x*64 + y
                            out_ap = bass.AP(
                                tensor=qkvT[jt].tensor,
                                offset=qkvT[jt].offset + 8 * nt,
                                ap=[qkvT[jt].ap[0], [1, 8], [64, 64]])
                        else:
                            out_ap = qkvT[jt][:, nt * 512:(nt + 1) * 512]
                        nc.vector.tensor_scalar_add(
                            out_ap, ps, bqkv_sb[:, jt:jt + 1])

        # ---------------- attention ----------------
        concatT = [persist.tile([128, N], BF16, name=f"concatT{fc}", tag=f"concatT{fc}")
                   for fc in range(4)]

        # pair list: 32 head-pairs; each pair = 2 heads sharing a qkvT tile
        pairs = [(half, s, hp)
                 for half in range(2) for s in range(NS) for hp in range(2)]

        # PSUM budget (8 banks): scores 3x[128,1024] (6) + LePE/transpose
        # shared slot (1) + AV output (1)
        with tc.tile_pool(name="sc_psum", bufs=3, space="PSUM") as sc_psum, \
             tc.tile_pool(name="lp_psum", bufs=1, space="PSUM") as lp_psum, \
             tc.tile_pool(name="oa_psum", bufs=1, space="PSUM") as oa_psum, \
             tc.tile_pool(name="att", bufs=4) as att, \
             tc.tile_pool(name="lepe_sb", bufs=4) as lepe_sb, \
             tc.tile_pool(name="norm_sb", bufs=5) as norm_sb:

            def emit_lepe(pi):
                """LePE for pair pi -> vlsb SBUF pair slab [128, 512].

                9 K=128 diagonal matmuls accumulating in PSUM; the center
                tap goes first (full range, start=True) so every element
                has has_written set; shifted taps write only their valid
                (rows x cols) window via 2-level APs."""
                half, s, hp = pairs[pi]
                tok0 = s * 512
                vt = qkvT[8 + half * 2 + hp]
                vsp = vt[:, tok0:tok0 + 512]
                vl = lp_psum.tile([128, 512], F32, tag="lps", name="vl")
                order = [4, 0, 1, 2, 3, 5, 6, 7, 8]
                for ki, k in enumerate(order):
                    dr, dc = k // 3 - 1, k % 3 - 1
                    r0 = max(0, -dr)
                    nr = 8 - abs(dr)
                    x0 = max(0, -dc)
                    nx = 64 - abs(dc)
                    o_off = r0 * 64 + x0
                    i_off = (r0 + dr) * 64 + (x0 + dc)
                    if dc == 0:
                        out_ap = vl[:, o_off:o_off + nr * 64]
                        in_ap = vsp[:, i_off:i_off + nr * 64]
                    else:
                        out_ap = bass.AP(
                            tensor=vl.tensor, offset=vl.offset + o_off,
                            ap=[vl.ap[0], [64, nr], [1, nx]])
                        in_ap = bass.AP(
                            tensor=vsp.tensor, offset=vsp.offset + i_off,
                            ap=[vsp.ap[0], [64, nr], [1, nx]])
                    nc.tensor.matmul(
                        out_ap, diags[half][k], in_ap,
                        start=(ki == 0), stop=(ki == len(order) - 1),
                        skip_group_check=True)
                # copy + bias -> SBUF bf16 (pair slab)
                vlsb = lepe_sb.tile([128, 512], BF16, tag="vlsb", name="vlsb")
                nc.vector.tensor_scalar_add(vlsb, vl, lepe_b[half])
                return vlsb

            def emit_vna(pi, vlsb):
                """Transpose pair slab + build pair v_nat_aug [128, 520]:
                chunk (jc, head) at cols (jc*2+head)*65, col 64 = ones."""
                vnp = lp_psum.tile([128, 512], BF16, tag="lps", name="vnp")
                for jc in range(4):
                    nc.tensor.transpose(
                        vnp[:, jc * 128:(jc + 1) * 128],
                        vlsb[:, jc * 128:(jc + 1) * 128], id128)
                vna = lepe_sb.tile([128, 520], BF16, tag="vna", name="vna")
                # one copy for both heads: in (jc, head, d) -> out chunks
                vna_data = bass.AP(
                    tensor=vna.tensor, offset=vna.offset,
                    ap=[vna.ap[0], [130, 4], [65, 2], [1, 64]])
                nc.vector.tensor_copy(vna_data, vnp)
                vna_ones = bass.AP(
                    tensor=vna.tensor, offset=vna.offset + 64,
                    ap=[vna.ap[0], [130, 4], [65, 2]])
                nc.vector.memset(vna_ones, 1.0)
                return vna

            def emit_pair_scores(pi):
                """scoresT + exp for BOTH heads of pair pi, emitted
                chunk-interleaved: h0 uses PE rows 0-63 and h1 rows 64-127,
                so adjacent matmuls run concurrently on disjoint row
                groups.  Returns (esb_h0, esb_h1)."""
                half, s, hp = pairs[pi]
                tok0 = s * 512
                jt_off = half * 2 + hp
                esbs = []
                qkss = []
                for hh in range(2):
                    pbase = hh * 64
                    qkss.append((
                        qkvT[jt_off][pbase:pbase + 64, tok0:tok0 + 512],
                        qkvT[4 + jt_off][pbase:pbase + 64, tok0:tok0 + 512]))
                    esbs.append(att.tile([128, 2048], BF16, tag="esb",
                                         name="esb"))
                for sh in range(2):
                    for hh in range(2):
                        qs, ks = qkss[hh]
                        sps = sc_psum.tile([128, 1024], F32, tag="sps",
                                           name="sps")
                        for jj in range(2):
                            jc = 2 * sh + jj
                            nc.tensor.matmul(
                                sps[:, jj * 512:(jj + 1) * 512],
                                ks[:, jc * 128:(jc + 1) * 128], qs,
                                start=True, stop=True)
                        # exp; no max subtraction needed (scores ~ N(0,1))
                        nc.scalar.activation(
                            esbs[hh][:, sh * 1024:(sh + 1) * 1024], sps,
                            mybir.ActivationFunctionType.Exp,
                            bias=0.0, scale=SCALE)
                return esbs

            def emit_av_norm(pi, hh, vna, esb):
                half, s, hp = pairs[pi]
                tok0 = s * 512
                jt_off = half * 2 + hp
                pbase = hh * 64
                # AV: outT_aug rows 0-63 = result, row 64 = denominator
                oa = oa_psum.tile([65, 512], F32, tag="oa", name="oa")
                for jc in range(4):
                    nc.tensor.matmul(
                        oa, vna[:, (jc * 2 + hh) * 65:(jc * 2 + hh) * 65 + 65],
                        esb[:, jc * 512:(jc + 1) * 512],
                        start=(jc == 0), stop=(jc == 3))
                # normalization: concatT <- out * (1/denom)
                osb = norm_sb.tile([64, 512], BF16, tag="osb", name="osb")
                nc.vector.tensor_copy(osb, oa[0:64, :])
                rec = norm_sb.tile([1, 512], F32, tag="rec", name="rec")
                nc.vector.reciprocal(rec, oa[64:65, :])
                denb = norm_sb.tile([64, 512], F32, tag="denb", name="denb")
                nc.gpsimd.partition_broadcast(denb, rec)
                cfc = concatT[jt_off]
                if half == 0:
                    out_ap = cfc[pbase:pbase + 64, tok0:tok0 + 512]
                else:
                    # scatter col-major stripe to row-major:
                    # local j = x*64 + y -> t = y*64 + 8s + x
                    out_ap = bass.AP(
                        tensor=cfc.tensor,
                        offset=cfc.offset + pbase * cfc.ap[0][0] + 8 * s,
                        ap=[[cfc.ap[0][0], 64], [1, 8], [64, 64]])
                nc.gpsimd.tensor_tensor(
                    out_ap, osb, denb, mybir.AluOpType.mult)

            # two independent streams (h-half pairs 0-15, v-half pairs
            # 16-31) interleaved so one stream's compute fills the other's
            # semaphore-latency bubbles; within each stream, LePE/
            # transposes for pair i+1 are software-pipelined between the
            # heads of pair i.
            streams = [list(range(0, 16)), list(range(16, 32))]
            vna_cur = []
            for st in (0, 1):
                vlsb0 = emit_lepe(streams[st][0])
                vna_cur.append(emit_vna(streams[st][0], vlsb0))
            nsteps = len(streams[0])
            for i in range(nsteps):
                p = [streams[0][i], streams[1][i]]
                nxt = [streams[st][i + 1] if i + 1 < nsteps else None
                       for st in (0, 1)]
                esbA = emit_pair_scores(p[0])
                vlsb_next = [None, None]
                if nxt[0] is not None:
                    vlsb_next[0] = emit_lepe(nxt[0])
                emit_av_norm(p[0], 0, vna_cur[0], esbA[0])
                esbB = emit_pair_scores(p[1])
                emit_av_norm(p[0], 1, vna_cur[0], esbA[1])
                if nxt[1] is not None:
                    vlsb_next[1] = emit_lepe(nxt[1])
                emit_av_norm(p[1], 0, vna_cur[1], esbB[0])
                if nxt[0] is not None:
                    vna_cur[0] = emit_vna(nxt[0], vlsb_next[0])
                emit_av_norm(p[1], 1, vna_cur[1], esbB[1])
                if nxt[1] is not None:
                    vna_cur[1] = emit_vna(nxt[1], vlsb_next[1])

        # ---------------- proj ----------------
        with tc.tile_pool(name="pj_psum", bufs=4, space="PSUM") as pj_psum, \
             tc.tile_pool(name="pj", bufs=5) as pj:
            for tt in range(32):
                ps = pj_psum.tile([128, C], F32, tag="pjps")
                for fc in range(4):
                    nc.tensor.matmul(
                        ps, concatT[fc][:, tt * 128:(tt + 1) * 128],
                        wprojT[fc],
                        start=(fc == 0), stop=False,
                        skip_group_check=True)
                # bias via K=1 ones matmul
                nc.tensor.matmul(
                    ps, ones_row, bproj_sb,
                    start=False, stop=True, skip_group_check=True)
                osb = pj.tile([128, C], F32, tag="pjout")
                if tt % 2 == 0:
                    nc.vector.tensor_copy(osb, ps)
                else:
                    nc.scalar.activation(
                        osb, ps, mybir.ActivationFunctionType.Copy)
                nc.sync.dma_start(
                    out=y_d[tt * 128:(tt + 1) * 128, :], in_=osb)


def _get_nc():
    if "nc" not in _CACHE:
        _CACHE["nc"] = _build_nc()
    return _CACHE["nc"]


def kernel(**inputs):
    x = np.asarray(inputs["x"], dtype=np.float32)
    names = {
        "wqkv": "Wqkv", "bqkv": "bqkv", "wproj": "Wproj", "bproj": "bproj",
        "lepe_h_w": "lepe_h_w", "lepe_h_b": "lepe_h_b",
        "lepe_v_w": "lepe_v_w", "lepe_v_b": "lepe_v_b",
    }
    shared = {k: np.ascontiguousarray(np.asarray(inputs[v], dtype=np.float32))
              for k, v in names.items()}
    nc = _get_nc()
    in_maps = []
    for b in range(B):
        m = dict(shared)
        m["x"] = np.ascontiguousarray(x[b])
        in_maps.append(m)
    res = bass_utils.run_bass_kernel_spmd(nc, in_maps, core_ids=list(range(B)))
    out = np.stack([res.results[b]["y"] for b in range(B)], axis=0)
    return out.astype(np.float32)


if __name__ == "__main__":
    rng = np.random.default_rng(0)
    ins = {
        "x": rng.standard_normal((B, N, C), dtype=np.float32),
        "Wqkv": rng.standard_normal((3 * C, C), dtype=np.float32) * C ** -0.5,
        "bqkv": np.zeros(3 * C, np.float32),
        "Wproj": rng.standard_normal((C, C), dtype=np.float32) * C ** -0.5,
        "bproj": np.zeros(C, np.float32),
        "lepe_h_w": rng.standard_normal((3, 3, 1, HD), dtype=np.float32) / 3,
        "lepe_h_b": np.zeros(HD, np.float32),
        "lepe_v_w": rng.standard_normal((3, 3, 1, HD), dtype=np.float32) / 3,
        "lepe_v_b": np.zeros(HD, np.float32),
        "H": np.int64(H), "W": np.int64(W),
    }
    out = kernel(**ins)
    print(out.shape, out.dtype)



# revision 3
# speedup vs baseline: 1.0047x; 1.0047x over previous
"""CSWin attention Trainium2 kernel (v2).

Shapes (hardcoded): B=8, H=W=64, N=4096, C=512, 8 heads (4 horizontal-stripe,
4 vertical-stripe), head_dim=64, stripe width SPLIT=8.

Sharding: data-parallel over batch B across the 8 NeuronCores (1 image/core).

Host staging (in kernel()): x is cast to bf16 and pre-transposed to xT
[C, N]; Wqkv/Wproj are pre-transposed+cast; the LePE diagonal tiles, the
identity, and all bias layouts are prebuilt on host.  This removes the
entire on-device transpose/cast preamble.

On-chip (per core, matmuls bf16 with fp32 PSUM):
  - qkvT [1536, 4096] = WqkvT @ xT, bias fused into the PSUM->SBUF copy
    (alternating DVE tensor_scalar_add / ScalarE Identity+bias).  v-half
    head channels written column-major so vertical stripes are contiguous.
  - attention: two interleaved streams of head-pairs (v-half stripes first,
    then h-half), software-pipelined one pair ahead:
      * LePE: 9 K=128 diagonal matmuls with shifted windows accumulating
        v + conv(v) in PSUM (lazy-zero sub-blocks, single group);
      * scoresT [k, q] per head, exp on ScalarE out of PSUM;
      * AV flipped: out[q, d] = sum_k E[k,q] v_lepe[k,d] with a ones
        column in the rhs so col 64 is the softmax denominator; 16
        matmuls of N=65 per head accumulate into one [128, 260] PSUM
        tile (4 q-blocks, lazy-zero);
      * normalization: DVE reciprocal [128,4] + per-partition-scalar
        multiplies into a token-major pair slab, PE transpose back to
        channel-major, copy into concatT.
  - proj: token-major PSUM matmuls from concatT + WprojT, bias added via
    DVE tensor_add with a host-broadcast bias tile, DMA out.
"""

import numpy as np

import concourse.bass as bass
import concourse.bacc as bacc
import concourse.mybir as mybir
from concourse import bass_utils
from concourse.tile import TileContext

F32 = mybir.dt.float32
BF16 = mybir.dt.bfloat16

B = 8
H = 64
W = 64
N = H * W          # 4096
C = 512
NH = 8             # heads
HD = 64            # head dim
SP = 8             # stripe width
NS = 8             # stripes per direction
SCALE = HD ** -0.5

_CACHE = {}


def _build_nc():
    nc = bacc.Bacc("TRN2", target_bir_lowering=False, debug=False)

    xT_d = nc.dram_tensor("xT", (C, N), BF16, kind="ExternalInput").ap()
    wqkvT_d = nc.dram_tensor("wqkvT", (C, 3 * C), BF16, kind="ExternalInput").ap()
    wprojT_d = nc.dram_tensor("wprojT", (C, C), BF16, kind="ExternalInput").ap()
    dg_d = nc.dram_tensor("dg", (128, 19 * 128), BF16, kind="ExternalInput").ap()
    bqkv_d = nc.dram_tensor("bqkv12", (128, 12), F32, kind="ExternalInput").ap()
    lepeb_d = nc.dram_tensor("lepeb", (128, 2), F32, kind="ExternalInput").ap()
    bprojb_d = nc.dram_tensor("bprojb", (128, C), F32, kind="ExternalInput").ap()
    y_d = nc.dram_tensor("y", (N, C), F32, kind="ExternalOutput").ap()

    with TileContext(nc) as tc:
        _emit(nc, tc, xT_d, wqkvT_d, wprojT_d, dg_d, bqkv_d, lepeb_d,
              bprojb_d, y_d)
    nc.compile()
    return nc


def _emit(nc, tc, xT_d, wqkvT_d, wprojT_d, dg_d, bqkv_d, lepeb_d,
          bprojb_d, y_d):
    import contextlib
    ctx = contextlib.ExitStack()
    with ctx:
        persist = ctx.enter_context(tc.tile_pool(name="persist", bufs=1))
        qkv_pool = ctx.enter_context(tc.tile_pool(name="qkvT", bufs=1))

        # ---------------- constants / weights (host-staged) ----------------
        wqkvT = [persist.tile([128, 3 * C], BF16, name=f"wqkvT{cc}", tag=f"wqkvT{cc}")
                 for cc in range(4)]
        for cc in range(4):
            nc.sync.dma_start(out=wqkvT[cc],
                              in_=wqkvT_d[cc * 128:(cc + 1) * 128, :])
        bqkv_sb = persist.tile([128, 12], F32, tag="bqkv")
        nc.sync.dma_start(out=bqkv_sb, in_=bqkv_d)

        wprojT = [persist.tile([128, C], BF16, name=f"wprojT{fc}", tag=f"wprojT{fc}")
                  for fc in range(4)]
        for fc in range(4):
            nc.sync.dma_start(out=wprojT[fc],
                              in_=wprojT_d[fc * 128:(fc + 1) * 128, :])
        dg = persist.tile([128, 19 * 128], BF16, tag="dg")
        nc.sync.dma_start(out=dg, in_=dg_d)
        id128 = dg[:, 18 * 128:19 * 128]

        def diag(half, k):
            return dg[:, (half * 9 + k) * 128:(half * 9 + k + 1) * 128]

        lepeb = persist.tile([128, 2], F32, tag="lepeb")
        nc.sync.dma_start(out=lepeb, in_=lepeb_d)
        bprojb = persist.tile([128, C], F32, tag="bprojb")
        nc.sync.dma_start(out=bprojb, in_=bprojb_d)

        # ---------------- P1: qkvT [1536, 4096] ----------------
        # qkvT tile jt holds channels [128*jt, 128*(jt+1)): jt 0-3 q, 4-7 k,
        # 8-11 v; within a group tiles 0-1 = h-half heads (row-major
        # tokens), 2-3 = v-half (column-major token order t' = x*64 + y).
        qkvT = [qkv_pool.tile([128, N], BF16, name=f"qkvT{jt}", tag=f"qkvT{jt}")
                for jt in range(12)]
        with tc.tile_pool(name="xT", bufs=1) as xT_pool:
            xT = [xT_pool.tile([128, N], BF16, name=f"xT{cc}", tag=f"xT{cc}") for cc in range(4)]
            for cc in range(4):
                for hf in range(2):
                    nc.sync.dma_start(
                        out=xT[cc][:, hf * 2048:(hf + 1) * 2048],
                        in_=xT_d[cc * 128:(cc + 1) * 128,
                                 hf * 2048:(hf + 1) * 2048])

            with tc.tile_pool(name="qkv_psum", bufs=6, space="PSUM") as qkv_psum:
                # v-half-related tiles first so attention can start early
                jts = [2, 6, 10, 3, 7, 11, 0, 4, 8, 1, 5, 9]
                for ji, jt in enumerate(jts):
                    vhalf = (jt % 4) >= 2
                    for nt in range(8):
                        ps = qkv_psum.tile([128, 512], F32, tag="qkvps")
                        for cc in range(4):
                            nc.tensor.matmul(
                                ps, wqkvT[cc][:, jt * 128:(jt + 1) * 128],
                                xT[cc][:, nt * 512:(nt + 1) * 512],
                                start=(cc == 0), stop=(cc == 3))
                        if vhalf:
                            # scatter token chunk (rows y in [8nt, 8nt+8),
                            # all x) into column-major: addr = x*64 + y
                            out_ap = bass.AP(
                                tensor=qkvT[jt].tensor,
                                offset=qkvT[jt].offset + 8 * nt,
                                ap=[qkvT[jt].ap[0], [1, 8], [64, 64]])
                        else:
                            out_ap = qkvT[jt][:, nt * 512:(nt + 1) * 512]
                        if (ji + nt) % 2 == 0:
                            nc.vector.tensor_scalar_add(
                                out_ap, ps, bqkv_sb[:, jt:jt + 1])
                        else:
                            nc.scalar.activation(
                                out_ap, ps,
                                mybir.ActivationFunctionType.Identity,
                                bias=bqkv_sb[:, jt:jt + 1], scale=1.0)

        # ---------------- P2: attention ----------------
        concatT = [persist.tile([128, N], BF16, name=f"concatT{fc}", tag=f"concatT{fc}")
                   for fc in range(4)]

        # stream st handles hp=st; v-half stripes first, then h-half
        streams = [[(1, s, st) for s in range(NS)] + [(0, s, st) for s in range(NS)]
                   for st in (0, 1)]

        # PSUM (8 banks): sps 2x[128,1024]f32 (4) + lp 1x[128,512] (1) +
        # oa 2x[128,260]f32 (2) + pt 1x[128,128]bf16 (1)
        with tc.tile_pool(name="sc_psum", bufs=2, space="PSUM") as sc_psum, \
             tc.tile_pool(name="lp_psum", bufs=1, space="PSUM") as lp_psum, \
             tc.tile_pool(name="oa_psum", bufs=2, space="PSUM") as oa_psum, \
             tc.tile_pool(name="pt_psum", bufs=1, space="PSUM") as pt_psum, \
             tc.tile_pool(name="att", bufs=4) as att, \
             tc.tile_pool(name="lepe_sb", bufs=2) as lepe_sb, \
             tc.tile_pool(name="vna_sb", bufs=2) as vna_sb, \
             tc.tile_pool(name="po_sb", bufs=2) as po_sb, \
             tc.tile_pool(name="rec_sb", bufs=2) as rec_sb:

            def emit_lepe(pr):
                """LePE for pair pr -> vlsb SBUF pair slab [128, 512]."""
                half, s, hp = pr
                tok0 = s * 512
                vt = qkvT[8 + half * 2 + hp]
                vsp = vt[:, tok0:tok0 + 512]
                vl = lp_psum.tile([128, 512], F32, tag="lps", name="vl")
                order = [4, 0, 1, 2, 3, 5, 6, 7, 8]
                for ki, k in enumerate(order):
                    dr, dc = k // 3 - 1, k % 3 - 1
                    r0 = max(0, -dr)
                    nr = 8 - abs(dr)
                    x0 = max(0, -dc)
                    nx = 64 - abs(dc)
                    o_off = r0 * 64 + x0
                    i_off = (r0 + dr) * 64 + (x0 + dc)
                    if dc == 0:
                        out_ap = vl[:, o_off:o_off + nr * 64]
                        in_ap = vsp[:, i_off:i_off + nr * 64]
                    else:
                        out_ap = bass.AP(
                            tensor=vl.tensor, offset=vl.offset + o_off,
                            ap=[vl.ap[0], [64, nr], [1, nx]])
                        in_ap = bass.AP(
                            tensor=vsp.tensor, offset=vsp.offset + i_off,
                            ap=[vsp.ap[0], [64, nr], [1, nx]])
                    nc.tensor.matmul(
                        out_ap, diag(half, k), in_ap,
                        start=(ki == 0), stop=(ki == len(order) - 1),
                        skip_group_check=True)
                vlsb = lepe_sb.tile([128, 512], BF16, tag="vlsb", name="vlsb")
                nc.scalar.activation(
                    vlsb, vl, mybir.ActivationFunctionType.Identity,
                    bias=lepeb[:, half:half + 1], scale=1.0)
                return vlsb

            def emit_vna(pr, vlsb):
                """Transpose pair slab + build pair v_nat_aug [128, 520]:
                chunk (jc, head) at cols (jc*2+head)*65, col 64 = ones."""
                vnp = lp_psum.tile([128, 512], BF16, tag="lps", name="vnp")
                for jc in range(4):
                    nc.tensor.transpose(
                        vnp[:, jc * 128:(jc + 1) * 128],
                        vlsb[:, jc * 128:(jc + 1) * 128], id128)
                vna = vna_sb.tile([128, 520], BF16, tag="vna", name="vna")
                vna_data = bass.AP(
                    tensor=vna.tensor, offset=vna.offset,
                    ap=[vna.ap[0], [130, 4], [65, 2], [1, 64]])
                nc.vector.tensor_copy(vna_data, vnp)
                vna_ones = bass.AP(
                    tensor=vna.tensor, offset=vna.offset + 64,
                    ap=[vna.ap[0], [130, 4], [65, 2]])
                nc.vector.memset(vna_ones, 1.0)
                return vna

            def emit_pair_scores(pr):
                """scoresT + exp for BOTH heads of pair pr (chunk-
                interleaved on disjoint PE row groups); returns esb x2."""
                half, s, hp = pr
                tok0 = s * 512
                jt_off = half * 2 + hp
                esbs = []
                qkss = []
                for hh in range(2):
                    pbase = hh * 64
                    qkss.append((
                        qkvT[jt_off][pbase:pbase + 64, tok0:tok0 + 512],
                        qkvT[4 + jt_off][pbase:pbase + 64, tok0:tok0 + 512]))
                    esbs.append(att.tile([128, 2048], BF16, tag="esb",
                                         name="esb"))
                for sh in range(2):
                    for hh in range(2):
                        qs, ks = qkss[hh]
                        sps = sc_psum.tile([128, 1024], F32, tag="sps",
                                           name="sps")
                        for jj in range(2):
                            jc = 2 * sh + jj
                            nc.tensor.matmul(
                                sps[:, jj * 512:(jj + 1) * 512],
                                ks[:, jc * 128:(jc + 1) * 128], qs,
                                start=True, stop=True)
                        # exp; no max subtraction needed (scores ~ N(0,1))
                        nc.scalar.activation(
                            esbs[hh][:, sh * 1024:(sh + 1) * 1024], sps,
                            mybir.ActivationFunctionType.Exp,
                            bias=0.0, scale=SCALE)
                return esbs

            def emit_avh(pr, hh, vna, esb, po, par):
                """AV (flipped) for head hh of pair pr: out[q, d] in oa
                [128, 4*65], then reciprocal + scale into po slab."""
                half, s, hp = pr
                oa = oa_psum.tile([128, 260], F32, tag="oa", name="oa")
                for qb in range(4):
                    for jc in range(4):
                        nc.tensor.matmul(
                            oa[:, qb * 65:qb * 65 + 65],
                            esb[:, jc * 512 + qb * 128:jc * 512 + qb * 128 + 128],
                            vna[:, (jc * 2 + hh) * 65:(jc * 2 + hh) * 65 + 65],
                            start=(qb == 0 and jc == 0),
                            stop=(qb == 3 and jc == 3),
                            skip_group_check=True)
                rec = rec_sb.tile([128, 4], F32, tag="rec", name="rec")
                den_ap = bass.AP(tensor=oa.tensor, offset=oa.offset + 64,
                                 ap=[oa.ap[0], [65, 4]])
                nc.vector.reciprocal(rec, den_ap)
                for qb in range(4):
                    dst = po[:, qb * 128 + hh * 64:qb * 128 + hh * 64 + 64]
                    src = oa[:, qb * 65:qb * 65 + 64]
                    if (qb + hh + par) % 2 == 0:
                        nc.vector.tensor_scalar_mul(dst, src, rec[:, qb:qb + 1])
                    else:
                        nc.scalar.activation(
                            dst, src, mybir.ActivationFunctionType.Copy,
                            scale=rec[:, qb:qb + 1])

            def emit_poout(pr, po, par):
                """Transpose po slab back to channel-major, write concatT."""
                half, s, hp = pr
                fc = (2 if half else 0) + hp
                cfc = concatT[fc]
                for qb in range(4):
                    pt = pt_psum.tile([128, 128], BF16, tag="pt", name="pt")
                    nc.tensor.transpose(pt, po[:, qb * 128:(qb + 1) * 128],
                                        id128)
                    if half == 0:
                        out_ap = cfc[:, s * 512 + qb * 128:s * 512 + (qb + 1) * 128]
                        in_ap = pt
                    else:
                        # local col j = xi*64 + y -> t = y*64 + 8s + 2qb + xi
                        out_ap = bass.AP(
                            tensor=cfc.tensor,
                            offset=cfc.offset + 8 * s + 2 * qb,
                            ap=[cfc.ap[0], [1, 2], [64, 64]])
                        in_ap = bass.AP(
                            tensor=pt.tensor, offset=pt.offset,
                            ap=[pt.ap[0], [64, 2], [1, 64]])
                    if (qb + par) % 2 == 0:
                        nc.vector.tensor_copy(out_ap, in_ap)
                    else:
                        nc.scalar.activation(
                            out_ap, in_ap,
                            mybir.ActivationFunctionType.Copy)

            # two streams interleaved; LePE/vna for pair i+1 pipelined
            # between the AV work of pair i; poout for stream1 deferred to
            # the start of the next step so its DVE chain has drained.
            vna_cur = []
            for st in (0, 1):
                vlsb0 = emit_lepe(streams[st][0])
                vna_cur.append(emit_vna(streams[st][0], vlsb0))
            nsteps = len(streams[0])
            pending = None      # (pair, po, par) awaiting poout
            for i in range(nsteps):
                p = [streams[0][i], streams[1][i]]
                nxt = [streams[st][i + 1] if i + 1 < nsteps else None
                       for st in (0, 1)]
                if pending is not None:
                    emit_poout(*pending)
                    pending = None
                esbA = emit_pair_scores(p[0])
                vlsb_next = [None, None]
                if nxt[0] is not None:
                    vlsb_next[0] = emit_lepe(nxt[0])
                poA = po_sb.tile([128, 512], BF16, tag="po", name="po")
                emit_avh(p[0], 0, vna_cur[0], esbA[0], poA, i)
                esbB = emit_pair_scores(p[1])
                emit_avh(p[0], 1, vna_cur[0], esbA[1], poA, i)
                if nxt[1] is not None:
                    vlsb_next[1] = emit_lepe(nxt[1])
                emit_poout(p[0], poA, i)
                poB = po_sb.tile([128, 512], BF16, tag="po", name="po")
                emit_avh(p[1], 0, vna_cur[1], esbB[0], poB, i + 1)
                if nxt[0] is not None:
                    vna_cur[0] = emit_vna(nxt[0], vlsb_next[0])
                emit_avh(p[1], 1, vna_cur[1], esbB[1], poB, i + 1)
                if nxt[1] is not None:
                    vna_cur[1] = emit_vna(nxt[1], vlsb_next[1])
                pending = (p[1], poB, i + 1)
            emit_poout(*pending)

        # ---------------- P3: proj ----------------
        with tc.tile_pool(name="pj_psum", bufs=4, space="PSUM") as pj_psum, \
             tc.tile_pool(name="pj", bufs=3) as pj:
            for tt in range(32):
                ps = pj_psum.tile([128, C], F32, tag="pjps")
                for fc in range(4):
                    nc.tensor.matmul(
                        ps, concatT[fc][:, tt * 128:(tt + 1) * 128],
                        wprojT[fc],
                        start=(fc == 0), stop=(fc == 3))
                osb = pj.tile([128, C], F32, tag="pjout")
                nc.vector.tensor_add(osb, ps, bprojb)
                nc.sync.dma_start(
                    out=y_d[tt * 128:(tt + 1) * 128, :], in_=osb)


def _get_nc():
    if "nc" not in _CACHE:
        _CACHE["nc"] = _build_nc()
    return _CACHE["nc"]


def _host_stage(inputs):
    import ml_dtypes
    bf16 = ml_dtypes.bfloat16
    wqkv = np.asarray(inputs["Wqkv"], np.float32)
    wproj = np.asarray(inputs["Wproj"], np.float32)
    bqkv = np.asarray(inputs["bqkv"], np.float32)
    bproj = np.asarray(inputs["bproj"], np.float32)
    lhw = np.asarray(inputs["lepe_h_w"], np.float32).reshape(9, HD)
    lvw = np.asarray(inputs["lepe_v_w"], np.float32).reshape(9, HD)
    lhb = np.asarray(inputs["lepe_h_b"], np.float32)
    lvb = np.asarray(inputs["lepe_v_b"], np.float32)

    wqkvT = np.ascontiguousarray(wqkv.T).astype(bf16)          # [C, 3C]
    wprojT = np.ascontiguousarray(wproj.T).astype(bf16)        # [C, C]
    bqkv12 = np.ascontiguousarray(bqkv.reshape(12, 128).T)     # [128, 12]
    bprojb = np.ascontiguousarray(
        np.broadcast_to(bproj, (128, C))).astype(np.float32)   # [128, C]
    lepeb = np.stack([np.tile(lhb, 2), np.tile(lvb, 2)], axis=1)
    lepeb = np.ascontiguousarray(lepeb).astype(np.float32)     # [128, 2]

    # diag tiles [128, 19*128]: (half, k) at col (half*9+k)*128; block 18
    # is the identity.  Center tap (k=4) has I added (v + conv(v)).
    dg = np.zeros((128, 19 * 128), np.float32)
    for half, w9 in ((0, lhw), (1, lvw)):
        for k in range(9):
            dr, dc = k // 3 - 1, k % 3 - 1
            if half == 0:
                wi = (dr + 1) * 3 + (dc + 1)
            else:
                wi = (dc + 1) * 3 + (dr + 1)
            vals = np.tile(w9[wi], 2)                          # [128]
            d = np.diag(vals)
            if k == 4:
                d = d + np.eye(128, dtype=np.float32)
            dg[:, (half * 9 + k) * 128:(half * 9 + k + 1) * 128] = d
    dg[:, 18 * 128:19 * 128] = np.eye(128, dtype=np.float32)
    dg = dg.astype(bf16)

    return {
        "wqkvT": wqkvT, "wprojT": wprojT, "dg": np.ascontiguousarray(dg),
        "bqkv12": bqkv12.astype(np.float32), "lepeb": lepeb,
        "bprojb": bprojb,
    }


def kernel(**inputs):
    import ml_dtypes
    bf16 = ml_dtypes.bfloat16
    x = np.asarray(inputs["x"], dtype=np.float32)
    shared = _host_stage(inputs)
    nc = _get_nc()
    in_maps = []
    for b in range(B):
        m = dict(shared)
        m["xT"] = np.ascontiguousarray(x[b].T).astype(bf16)
        in_maps.append(m)
    res = bass_utils.run_bass_kernel_spmd(nc, in_maps, core_ids=list(range(B)))
    out = np.stack([res.results[b]["y"] for b in range(B)], axis=0)
    return out.astype(np.float32)


if __name__ == "__main__":
    rng = np.random.default_rng(0)
    ins = {
        "x": rng.standard_normal((B, N, C), dtype=np.float32),
        "Wqkv": rng.standard_normal((3 * C, C), dtype=np.float32) * C ** -0.5,
        "bqkv": np.zeros(3 * C, np.float32),
        "Wproj": rng.standard_normal((C, C), dtype=np.float32) * C ** -0.5,
        "bproj": np.zeros(C, np.float32),
        "lepe_h_w": rng.standard_normal((3, 3, 1, HD), dtype=np.float32) / 3,
        "lepe_h_b": np.zeros(HD, np.float32),
        "lepe_v_w": rng.standard_normal((3, 3, 1, HD), dtype=np.float32) / 3,
        "lepe_v_b": np.zeros(HD, np.float32),
        "H": np.int64(H), "W": np.int64(W),
    }
    out = kernel(**ins)
    print(out.shape, out.dtype)


# revision 5
# speedup vs baseline: 1.1121x; 1.1069x over previous
"""CSWin attention Trainium2 kernel (v2).

Shapes (hardcoded): B=8, H=W=64, N=4096, C=512, 8 heads (4 horizontal-stripe,
4 vertical-stripe), head_dim=64, stripe width SPLIT=8.

Sharding: data-parallel over batch B across the 8 NeuronCores (1 image/core).

Host staging (in kernel()): x is cast to bf16 and pre-transposed to xT
[C, N]; Wqkv/Wproj are pre-transposed+cast; the LePE diagonal tiles, the
identity, and all bias layouts are prebuilt on host.  This removes the
entire on-device transpose/cast preamble.

On-chip (per core, matmuls bf16 with fp32 PSUM):
  - qkvT [1536, 4096] = WqkvT @ xT, bias fused into the PSUM->SBUF copy
    (alternating DVE tensor_scalar_add / ScalarE Identity+bias).  v-half
    head channels written column-major so vertical stripes are contiguous.
  - attention: two interleaved streams of head-pairs (v-half stripes first,
    then h-half), software-pipelined one pair ahead:
      * LePE: 9 K=128 diagonal matmuls with shifted windows accumulating
        v + conv(v) in PSUM (lazy-zero sub-blocks, single group);
      * scoresT [k, q] per head, exp on ScalarE out of PSUM;
      * AV flipped: out[q, d] = sum_k E[k,q] v_lepe[k,d] with a ones
        column in the rhs so col 64 is the softmax denominator; 16
        matmuls of N=65 per head accumulate into one [128, 260] PSUM
        tile (4 q-blocks, lazy-zero);
      * normalization: DVE reciprocal [128,4] + per-partition-scalar
        multiplies into a token-major pair slab, PE transpose back to
        channel-major, copy into concatT.
  - proj: token-major PSUM matmuls from concatT + WprojT, bias added via
    DVE tensor_add with a host-broadcast bias tile, DMA out.
"""

import numpy as np

import concourse.bass as bass
import concourse.bacc as bacc
import concourse.mybir as mybir
from concourse import bass_utils
from concourse.tile import TileContext

F32 = mybir.dt.float32
BF16 = mybir.dt.bfloat16

B = 8
H = 64
W = 64
N = H * W          # 4096
C = 512
NH = 8             # heads
HD = 64            # head dim
SP = 8             # stripe width
NS = 8             # stripes per direction
SCALE = HD ** -0.5

_CACHE = {}


def _build_nc():
    nc = bacc.Bacc("TRN2", target_bir_lowering=False, debug=False)

    xT_d = nc.dram_tensor("xT", (C, N), BF16, kind="ExternalInput").ap()
    wqkvT_d = nc.dram_tensor("wqkvT", (C, 3 * C), BF16, kind="ExternalInput").ap()
    wprojT_d = nc.dram_tensor("wprojT", (C, C), BF16, kind="ExternalInput").ap()
    dg_d = nc.dram_tensor("dg", (128, 19 * 128), BF16, kind="ExternalInput").ap()
    bqkv_d = nc.dram_tensor("bqkv12", (128, 12), F32, kind="ExternalInput").ap()
    lepeb_d = nc.dram_tensor("lepeb", (128, 2), F32, kind="ExternalInput").ap()
    bprojb_d = nc.dram_tensor("bprojb", (128, C), F32, kind="ExternalInput").ap()
    y_d = nc.dram_tensor("y", (N, C), F32, kind="ExternalOutput").ap()

    with TileContext(nc) as tc:
        _emit(nc, tc, xT_d, wqkvT_d, wprojT_d, dg_d, bqkv_d, lepeb_d,
              bprojb_d, y_d)
    nc.compile()
    return nc


def _emit(nc, tc, xT_d, wqkvT_d, wprojT_d, dg_d, bqkv_d, lepeb_d,
          bprojb_d, y_d):
    import contextlib
    ctx = contextlib.ExitStack()
    with ctx:
        persist = ctx.enter_context(tc.tile_pool(name="persist", bufs=1))
        qkv_pool = ctx.enter_context(tc.tile_pool(name="qkvT", bufs=1))

        # ---------------- constants / weights (host-staged) ----------------
        wqkvT = [persist.tile([128, 3 * C], BF16, name=f"wqkvT{cc}", tag=f"wqkvT{cc}")
                 for cc in range(4)]
        for cc in range(4):
            for g in range(3):
                nc.sync.dma_start(
                    out=wqkvT[cc][:, g * 512:(g + 1) * 512],
                    in_=wqkvT_d[cc * 128:(cc + 1) * 128, g * 512:(g + 1) * 512])
        bqkv_sb = persist.tile([128, 12], F32, tag="bqkv")
        nc.sync.dma_start(out=bqkv_sb, in_=bqkv_d)

        wprojT = [persist.tile([128, C], BF16, name=f"wprojT{fc}", tag=f"wprojT{fc}")
                  for fc in range(4)]
        for fc in range(4):
            nc.sync.dma_start(out=wprojT[fc],
                              in_=wprojT_d[fc * 128:(fc + 1) * 128, :])
        dg = persist.tile([128, 19 * 128], BF16, tag="dg")
        nc.sync.dma_start(out=dg, in_=dg_d)
        id128 = dg[:, 18 * 128:19 * 128]

        def diag(half, k):
            return dg[:, (half * 9 + k) * 128:(half * 9 + k + 1) * 128]

        lepeb = persist.tile([128, 2], F32, tag="lepeb")
        nc.sync.dma_start(out=lepeb, in_=lepeb_d)
        bprojb = persist.tile([128, C], F32, tag="bprojb")
        nc.sync.dma_start(out=bprojb, in_=bprojb_d)

        # ---------------- P1: qkvT [1536, 4096] ----------------
        # qkvT tile jt holds channels [128*jt, 128*(jt+1)): jt 0-3 q, 4-7 k,
        # 8-11 v; within a group tiles 0-1 = h-half heads (row-major
        # tokens), 2-3 = v-half (column-major token order t' = x*64 + y).
        qkvT = [qkv_pool.tile([128, N], BF16, name=f"qkvT{jt}", tag=f"qkvT{jt}")
                for jt in range(12)]
        with tc.tile_pool(name="xT", bufs=1) as xT_pool:
            xT = [xT_pool.tile([128, N], BF16, name=f"xT{cc}", tag=f"xT{cc}") for cc in range(4)]
            for hf in range(4):
                for cc in range(4):
                    nc.sync.dma_start(
                        out=xT[cc][:, hf * 1024:(hf + 1) * 1024],
                        in_=xT_d[cc * 128:(cc + 1) * 128,
                                 hf * 1024:(hf + 1) * 1024])

            with tc.tile_pool(name="qkv_psum", bufs=6, space="PSUM") as qkv_psum:
                # v-half-related tiles first so attention can start early
                jts = [2, 6, 10, 3, 7, 11, 0, 4, 8, 1, 5, 9]
                for ji, jt in enumerate(jts):
                    vhalf = (jt % 4) >= 2
                    for nt in range(8):
                        ps = qkv_psum.tile([128, 512], F32, tag="qkvps")
                        for cc in range(4):
                            nc.tensor.matmul(
                                ps, wqkvT[cc][:, jt * 128:(jt + 1) * 128],
                                xT[cc][:, nt * 512:(nt + 1) * 512],
                                start=(cc == 0), stop=(cc == 3))
                        if vhalf:
                            # scatter token chunk (rows y in [8nt, 8nt+8),
                            # all x) into column-major: addr = x*64 + y
                            out_ap = bass.AP(
                                tensor=qkvT[jt].tensor,
                                offset=qkvT[jt].offset + 8 * nt,
                                ap=[qkvT[jt].ap[0], [1, 8], [64, 64]])
                        else:
                            out_ap = qkvT[jt][:, nt * 512:(nt + 1) * 512]
                        if (ji + nt) % 2 == 0:
                            nc.vector.tensor_scalar_add(
                                out_ap, ps, bqkv_sb[:, jt:jt + 1])
                        else:
                            nc.scalar.activation(
                                out_ap, ps,
                                mybir.ActivationFunctionType.Identity,
                                bias=bqkv_sb[:, jt:jt + 1], scale=1.0)

        # ---------------- P2: attention ----------------
        concatT = [persist.tile([128, N], BF16, name=f"concatT{fc}", tag=f"concatT{fc}")
                   for fc in range(4)]

        # stream st handles hp=st; v-half stripes first, then h-half
        streams = [[(1, s, st) for s in range(NS)] + [(0, s, st) for s in range(NS)]
                   for st in (0, 1)]

        # PSUM (8 banks): sps 2x[128,1024]f32 (4) + lp 1x[128,512] (1) +
        # oa 2x[128,260]f32 (2) + pt 1x[128,128]bf16 (1)
        with tc.tile_pool(name="sc_psum", bufs=2, space="PSUM") as sc_psum, \
             tc.tile_pool(name="lp_psum", bufs=1, space="PSUM") as lp_psum, \
             tc.tile_pool(name="oa_psum", bufs=2, space="PSUM") as oa_psum, \
             tc.tile_pool(name="pt_psum", bufs=1, space="PSUM") as pt_psum, \
             tc.tile_pool(name="att", bufs=4) as att, \
             tc.tile_pool(name="lepe_sb", bufs=2) as lepe_sb, \
             tc.tile_pool(name="vna_sb", bufs=2) as vna_sb, \
             tc.tile_pool(name="po_sb", bufs=2) as po_sb, \
             tc.tile_pool(name="rec_sb", bufs=2) as rec_sb:

            def emit_lepe(pr):
                """LePE for pair pr -> vlsb SBUF pair slab [128, 512]."""
                half, s, hp = pr
                tok0 = s * 512
                vt = qkvT[8 + half * 2 + hp]
                vsp = vt[:, tok0:tok0 + 512]
                vl = lp_psum.tile([128, 512], F32, tag="lps", name="vl")
                order = [4, 0, 1, 2, 3, 5, 6, 7, 8]
                for ki, k in enumerate(order):
                    dr, dc = k // 3 - 1, k % 3 - 1
                    r0 = max(0, -dr)
                    nr = 8 - abs(dr)
                    x0 = max(0, -dc)
                    nx = 64 - abs(dc)
                    o_off = r0 * 64 + x0
                    i_off = (r0 + dr) * 64 + (x0 + dc)
                    if dc == 0:
                        out_ap = vl[:, o_off:o_off + nr * 64]
                        in_ap = vsp[:, i_off:i_off + nr * 64]
                    else:
                        out_ap = bass.AP(
                            tensor=vl.tensor, offset=vl.offset + o_off,
                            ap=[vl.ap[0], [64, nr], [1, nx]])
                        in_ap = bass.AP(
                            tensor=vsp.tensor, offset=vsp.offset + i_off,
                            ap=[vsp.ap[0], [64, nr], [1, nx]])
                    nc.tensor.matmul(
                        out_ap, diag(half, k), in_ap,
                        start=(ki == 0), stop=(ki == len(order) - 1),
                        skip_group_check=True)
                return (vl, half)

            def emit_vna(pr, vlh):
                """vl PSUM -> vlsb (DVE, +bias), transpose, build pair
                v_nat_aug [128, 520]: chunk (jc, head) at cols
                (jc*2+head)*65, col 64 = ones."""
                vl, half = vlh
                vlsb = lepe_sb.tile([128, 512], BF16, tag="vlsb", name="vlsb")
                nc.vector.tensor_scalar_add(vlsb, vl, lepeb[:, half:half + 1])
                vnp = lp_psum.tile([128, 512], BF16, tag="lps", name="vnp")
                for jc in range(4):
                    nc.tensor.transpose(
                        vnp[:, jc * 128:(jc + 1) * 128],
                        vlsb[:, jc * 128:(jc + 1) * 128], id128)
                vna = vna_sb.tile([128, 520], BF16, tag="vna", name="vna")
                vna_data = bass.AP(
                    tensor=vna.tensor, offset=vna.offset,
                    ap=[vna.ap[0], [130, 4], [65, 2], [1, 64]])
                nc.vector.tensor_copy(vna_data, vnp)
                vna_ones = bass.AP(
                    tensor=vna.tensor, offset=vna.offset + 64,
                    ap=[vna.ap[0], [130, 4], [65, 2]])
                nc.gpsimd.memset(vna_ones, 1.0)
                return vna

            def emit_pair_scores(pr):
                """scoresT + exp for BOTH heads of pair pr (chunk-
                interleaved on disjoint PE row groups); returns esb x2."""
                half, s, hp = pr
                tok0 = s * 512
                jt_off = half * 2 + hp
                esbs = []
                qkss = []
                for hh in range(2):
                    pbase = hh * 64
                    qkss.append((
                        qkvT[jt_off][pbase:pbase + 64, tok0:tok0 + 512],
                        qkvT[4 + jt_off][pbase:pbase + 64, tok0:tok0 + 512]))
                    esbs.append(att.tile([128, 2048], BF16, tag="esb",
                                         name="esb"))
                for sh in range(2):
                    for hh in range(2):
                        qs, ks = qkss[hh]
                        sps = sc_psum.tile([128, 1024], F32, tag="sps",
                                           name="sps")
                        for jj in range(2):
                            jc = 2 * sh + jj
                            nc.tensor.matmul(
                                sps[:, jj * 512:(jj + 1) * 512],
                                ks[:, jc * 128:(jc + 1) * 128], qs,
                                start=True, stop=True)
                        # exp; no max subtraction needed (scores ~ N(0,1))
                        nc.scalar.activation(
                            esbs[hh][:, sh * 1024:(sh + 1) * 1024], sps,
                            mybir.ActivationFunctionType.Exp,
                            bias=0.0, scale=SCALE)
                return esbs

            def emit_avh(pr, hh, vna, esb, po, par):
                """AV (flipped) for head hh of pair pr: out[q, d] in oa
                [128, 4*65], then reciprocal + scale into po slab."""
                half, s, hp = pr
                oa = oa_psum.tile([128, 260], F32, tag="oa", name="oa")
                for qb in range(4):
                    for jc in range(4):
                        nc.tensor.matmul(
                            oa[:, qb * 65:qb * 65 + 65],
                            esb[:, jc * 512 + qb * 128:jc * 512 + qb * 128 + 128],
                            vna[:, (jc * 2 + hh) * 65:(jc * 2 + hh) * 65 + 65],
                            start=(qb == 0 and jc == 0),
                            stop=(qb == 3 and jc == 3),
                            skip_group_check=True)
                rec = rec_sb.tile([128, 4], F32, tag="rec", name="rec")
                den_ap = bass.AP(tensor=oa.tensor, offset=oa.offset + 64,
                                 ap=[oa.ap[0], [65, 4]])
                nc.vector.reciprocal(rec, den_ap)
                for qb in range(4):
                    dst = po[:, qb * 128 + hh * 64:qb * 128 + hh * 64 + 64]
                    src = oa[:, qb * 65:qb * 65 + 64]
                    nc.vector.tensor_scalar_mul(dst, src, rec[:, qb:qb + 1])

            def emit_poout(pr, po, par):
                """Transpose po slab back to channel-major, write concatT."""
                half, s, hp = pr
                fc = (2 if half else 0) + hp
                cfc = concatT[fc]
                for qb in range(4):
                    pt = pt_psum.tile([128, 128], BF16, tag="pt", name="pt")
                    nc.tensor.transpose(pt, po[:, qb * 128:(qb + 1) * 128],
                                        id128)
                    if half == 0:
                        out_ap = cfc[:, s * 512 + qb * 128:s * 512 + (qb + 1) * 128]
                        in_ap = pt
                    else:
                        # local col j = xi*64 + y -> t = y*64 + 8s + 2qb + xi
                        out_ap = bass.AP(
                            tensor=cfc.tensor,
                            offset=cfc.offset + 8 * s + 2 * qb,
                            ap=[cfc.ap[0], [1, 2], [64, 64]])
                        in_ap = bass.AP(
                            tensor=pt.tensor, offset=pt.offset,
                            ap=[pt.ap[0], [64, 2], [1, 64]])
                    nc.vector.tensor_copy(out_ap, in_ap)

            # two streams interleaved; LePE/vna for pair i+1 pipelined
            # between the AV work of pair i; poout for stream1 deferred to
            # the start of the next step so its DVE chain has drained.
            vna_cur = []
            for st in (0, 1):
                vlh0 = emit_lepe(streams[st][0])
                vna_cur.append(emit_vna(streams[st][0], vlh0))
            nsteps = len(streams[0])
            pending = None      # (pair, po, par) awaiting poout
            for i in range(nsteps):
                p = [streams[0][i], streams[1][i]]
                nxt = [streams[st][i + 1] if i + 1 < nsteps else None
                       for st in (0, 1)]
                if pending is not None:
                    emit_poout(*pending)
                    pending = None
                esbA = emit_pair_scores(p[0])
                vlsb_next = [None, None]
                if nxt[0] is not None:
                    vlsb_next[0] = emit_lepe(nxt[0])
                poA = po_sb.tile([128, 512], BF16, tag="po", name="po")
                emit_avh(p[0], 0, vna_cur[0], esbA[0], poA, i)
                esbB = emit_pair_scores(p[1])
                emit_avh(p[0], 1, vna_cur[0], esbA[1], poA, i)
                if nxt[1] is not None:
                    vlsb_next[1] = emit_lepe(nxt[1])
                emit_poout(p[0], poA, i)
                poB = po_sb.tile([128, 512], BF16, tag="po", name="po")
                emit_avh(p[1], 0, vna_cur[1], esbB[0], poB, i + 1)
                if nxt[0] is not None:
                    vna_cur[0] = emit_vna(nxt[0], vlsb_next[0])
                emit_avh(p[1], 1, vna_cur[1], esbB[1], poB, i + 1)
                if nxt[1] is not None:
                    vna_cur[1] = emit_vna(nxt[1], vlsb_next[1])
                pending = (p[1], poB, i + 1)
            emit_poout(*pending)

        # ---------------- P3: proj ----------------
        with tc.tile_pool(name="pj_psum", bufs=4, space="PSUM") as pj_psum, \
             tc.tile_pool(name="pj", bufs=3) as pj:
            for tt in range(32):
                ps = pj_psum.tile([128, C], F32, tag="pjps")
                for fc in range(4):
                    nc.tensor.matmul(
                        ps, concatT[fc][:, tt * 128:(tt + 1) * 128],
                        wprojT[fc],
                        start=(fc == 0), stop=(fc == 3))
                osb = pj.tile([128, C], F32, tag="pjout")
                nc.vector.tensor_add(osb, ps, bprojb)
                nc.sync.dma_start(
                    out=y_d[tt * 128:(tt + 1) * 128, :], in_=osb)


def _get_nc():
    if "nc" not in _CACHE:
        _CACHE["nc"] = _build_nc()
    return _CACHE["nc"]


def _host_stage(inputs):
    import ml_dtypes
    bf16 = ml_dtypes.bfloat16
    wqkv = np.asarray(inputs["Wqkv"], np.float32)
    wproj = np.asarray(inputs["Wproj"], np.float32)
    bqkv = np.asarray(inputs["bqkv"], np.float32)
    bproj = np.asarray(inputs["bproj"], np.float32)
    lhw = np.asarray(inputs["lepe_h_w"], np.float32).reshape(9, HD)
    lvw = np.asarray(inputs["lepe_v_w"], np.float32).reshape(9, HD)
    lhb = np.asarray(inputs["lepe_h_b"], np.float32)
    lvb = np.asarray(inputs["lepe_v_b"], np.float32)

    wqkvT = np.ascontiguousarray(wqkv.T).astype(bf16)          # [C, 3C]
    wprojT = np.ascontiguousarray(wproj.T).astype(bf16)        # [C, C]
    bqkv12 = np.ascontiguousarray(bqkv.reshape(12, 128).T)     # [128, 12]
    bprojb = np.ascontiguousarray(
        np.broadcast_to(bproj, (128, C))).astype(np.float32)   # [128, C]
    lepeb = np.stack([np.tile(lhb, 2), np.tile(lvb, 2)], axis=1)
    lepeb = np.ascontiguousarray(lepeb).astype(np.float32)     # [128, 2]

    # diag tiles [128, 19*128]: (half, k) at col (half*9+k)*128; block 18
    # is the identity.  Center tap (k=4) has I added (v + conv(v)).
    dg = np.zeros((128, 19 * 128), np.float32)
    for half, w9 in ((0, lhw), (1, lvw)):
        for k in range(9):
            dr, dc = k // 3 - 1, k % 3 - 1
            if half == 0:
                wi = (dr + 1) * 3 + (dc + 1)
            else:
                wi = (dc + 1) * 3 + (dr + 1)
            vals = np.tile(w9[wi], 2)                          # [128]
            d = np.diag(vals)
            if k == 4:
                d = d + np.eye(128, dtype=np.float32)
            dg[:, (half * 9 + k) * 128:(half * 9 + k + 1) * 128] = d
    dg[:, 18 * 128:19 * 128] = np.eye(128, dtype=np.float32)
    dg = dg.astype(bf16)

    return {
        "wqkvT": wqkvT, "wprojT": wprojT, "dg": np.ascontiguousarray(dg),
        "bqkv12": bqkv12.astype(np.float32), "lepeb": lepeb,
        "bprojb": bprojb,
    }


def kernel(**inputs):
    import ml_dtypes
    bf16 = ml_dtypes.bfloat16
    x = np.asarray(inputs["x"], dtype=np.float32)
    shared = _host_stage(inputs)
    nc = _get_nc()
    in_maps = []
    for b in range(B):
        m = dict(shared)
        m["xT"] = np.ascontiguousarray(x[b].T).astype(bf16)
        in_maps.append(m)
    res = bass_utils.run_bass_kernel_spmd(nc, in_maps, core_ids=list(range(B)))
    out = np.stack([res.results[b]["y"] for b in range(B)], axis=0)
    return out.astype(np.float32)


if __name__ == "__main__":
    rng = np.random.default_rng(0)
    ins = {
        "x": rng.standard_normal((B, N, C), dtype=np.float32),
        "Wqkv": rng.standard_normal((3 * C, C), dtype=np.float32) * C ** -0.5,
        "bqkv": np.zeros(3 * C, np.float32),
        "Wproj": rng.standard_normal((C, C), dtype=np.float32) * C ** -0.5,
        "bproj": np.zeros(C, np.float32),
        "lepe_h_w": rng.standard_normal((3, 3, 1, HD), dtype=np.float32) / 3,
        "lepe_h_b": np.zeros(HD, np.float32),
        "lepe_v_w": rng.standard_normal((3, 3, 1, HD), dtype=np.float32) / 3,
        "lepe_v_b": np.zeros(HD, np.float32),
        "H": np.int64(H), "W": np.int64(W),
    }
    out = kernel(**ins)
    print(out.shape, out.dtype)
